# revision 12
# baseline (speedup 1.0000x reference)
"""Global-attention kernel for [8, 384, 32, 32] ConvAttention on 8 trn2 cores.

Math (per reference): tokens over B*H*W = 8192 positions, C = 384 channels
split as V/K/Q of 128 each; out = softmax(Q K^T / sqrt(128)) V, re-laid as
[B, 128, H, W].

Sharding: core c owns the 1024 query tokens of batch c (token n = b*1024+hw,
so batch == contiguous token block). K/V are replicated. Each core computes
its row block of the attention entirely locally; no collectives.

On-core layout: channel-major ([d, token]) everywhere, S^T formulation:
for each kv chunk j (128 tokens), S^T_j = K_j^T Q in PSUM (3 rotating
tiles), exp to bf16, then O^T += V_j^T E_j on PE, accumulated in two
half-tiles (cols 0:512 / 512:1024) so the first half drains one matmul
early.

Q/K are bf16 (halves input DMA vs f32; verified ~1e-2 rel err), V fp16.
The PE stream is the bottleneck (QK + AV = 2048 rows/chunk at 1 row/cycle,
2.4 GHz); every 4th chunk's exp runs as a Schraudolph bit-trick on Pool
(mostly) or DVE so the ACT engine (1038ns/exp vs 854ns chunk period) never
falls more than ~550ns behind and the PE never stalls on an e-tile or
PSUM-slot rotation.

Softmax denominator: bf16 adds on DVE into parity regions of rs2; odd
region folds (ones-matmul, partition-broadcast) after chunk 61, even after
62, and chunk 63's e folds directly, all before the last two AV matmuls so
the reciprocal (DVE) starts 426ns before the PE drains. Normalize+store is
pipelined in 4 query-column slices across DVE/Pool with 4 output DMAs.
"""

import math

import numpy as np
import ml_dtypes

import concourse.bass as bass
import concourse.tile as tile
from concourse import bacc, mybir
from concourse.alu_op_type import AluOpType
from concourse.bass_utils import run_bass_kernel_spmd

N_CORES = 8
B, C, H, W = 8, 384, 32, 32
HW = H * W            # 1024 tokens per batch == per core
N = B * HW            # 8192 total tokens
D = 128               # key/value width
NCHUNK = N // 128     # 64 kv chunks of 128 tokens
SCALE = 1.0 / math.sqrt(D)
F32 = mybir.dt.float32
F32R = mybir.dt.float32r
F16 = mybir.dt.float16
BF16 = mybir.dt.bfloat16
I16 = mybir.dt.int16

# Schraudolph exp (bf16 bit trick): i16 = x*a + b truncated to int16; the
# bit pattern read as bf16 approximates exp(x*SCALE) within ~3.5%.
A7S = float((1 << 7) / math.log(2.0) * SCALE)
B7 = float(127 * (1 << 7) - 6)

# Every 4th chunk's exp runs off the ACT engine (period-4 pattern keeps the
# ACT backlog bounded). GPSIMD cannot read PSUM on TRN2, so all of these
# run on DVE; to compensate, ~1 in 5 denominator adds (SBUF-only) moves to
# Pool.
SCHR = frozenset(c for c in range(NCHUNK) if c % 4 == 1)  # 16 chunks
POOL_RS = frozenset(c for c in range(2, 55) if c % 5 == 2)  # 11 chunks

N_WARMUP_MM = 13  # PE matmuls filling the pstate-ramp window while DMAs land


def _build_nc():
    nc = bacc.Bacc(
        "TRN2", target_bir_lowering=False, debug=False, num_devices=N_CORES
    )
    qT = nc.dram_tensor("qT", [D, HW], BF16, kind="ExternalInput").ap()
    kT00 = nc.dram_tensor("kT00", [D, D], BF16, kind="ExternalInput").ap()
    kT = nc.dram_tensor("kT", [D, N], BF16, kind="ExternalInput").ap()
    vt16 = nc.dram_tensor("vt16", [D, N], F16, kind="ExternalInput").ap()
    oT = nc.dram_tensor("oT", [D, HW], F32, kind="ExternalOutput").ap()

    with tile.TileContext(nc) as tc:
        with (
            tc.tile_pool(name="persist", bufs=1) as persist,
            tc.tile_pool(name="etile", bufs=10) as epool,
            tc.tile_pool(name="spsum", bufs=3, space="PSUM") as spsum,
            tc.tile_pool(name="apsum", bufs=1, space="PSUM") as apsum,
        ):
            # --- SBUF persistents ---
            qT_sb = persist.tile([D, HW], BF16, tag="qT_sb")
            kT00_sb = persist.tile([D, D], BF16, tag="kT00_sb")
            kT_sb = [
                persist.tile([D, HW], BF16, tag=f"kT{i}", name=f"kT_sb{i}")
                for i in range(8)
            ]
            vt_sb = [
                persist.tile([D, HW], F16, tag=f"vt{i}", name=f"vt_sb{i}")
                for i in range(8)
            ]
            ones16 = persist.tile([D, D], BF16, tag="ones16")
            # Denominator partials: region 0 = even chunks, 1 = odd.
            rs2 = persist.tile([D, 2 * HW], BF16, tag="rs2")
            warm_sb = persist.tile([D, 256], F32, tag="warm_sb")

            nc.gpsimd.memset(warm_sb[:], 0.5)
            nc.gpsimd.memset(ones16[:], 1.0)

            # --- input DMAs. The HWDGE config stage serializes at ~625ns
            # per DMA, so order by first-use and split the head of V so the
            # first AV isn't stuck behind a full-block transfer.
            nc.sync.dma_start(out=kT00_sb[:], in_=kT00[:])
            nc.sync.dma_start(out=qT_sb[:, 0:512], in_=qT[:, 0:512])
            nc.sync.dma_start(out=qT_sb[:, 512:1024], in_=qT[:, 512:1024])
            nc.sync.dma_start(out=kT_sb[0][:], in_=kT[:, 0:HW])
            nc.sync.dma_start(out=vt_sb[0][:, 0:128], in_=vt16[:, 0:128])
            nc.sync.dma_start(out=vt_sb[0][:, 128:256], in_=vt16[:, 128:256])
            nc.sync.dma_start(out=vt_sb[0][:, 256:1024], in_=vt16[:, 256:1024])
            for i in (1, 3, 5):
                nc.sync.dma_start(
                    out=kT_sb[i][:], in_=kT[:, i * HW : (i + 1) * HW]
                )
                nc.sync.dma_start(
                    out=kT_sb[i + 1][:], in_=kT[:, (i + 1) * HW : (i + 2) * HW]
                )
                nc.sync.dma_start(
                    out=vt_sb[i][:], in_=vt16[:, i * HW : (i + 1) * HW]
                )
                nc.sync.dma_start(
                    out=vt_sb[i + 1][:], in_=vt16[:, (i + 1) * HW : (i + 2) * HW]
                )
            nc.sync.dma_start(out=kT_sb[7][:], in_=kT[:, 7 * HW : 8 * HW])
            nc.sync.dma_start(out=vt_sb[7][:], in_=vt16[:, 7 * HW : 8 * HW])

            # --- PE warmup: occupy the sub-3us pstate window while the
            # first input DMAs land; results unused.
            wm_ps = spsum.tile([D, HW], F32, tag="s", name="warm_ps")
            for i in range(N_WARMUP_MM):
                nc.tensor.matmul(
                    wm_ps[:, 0:256],
                    warm_sb[:, 0:128].bitcast(F32R),
                    warm_sb[:].bitcast(F32R),
                    start=True,
                    stop=True,
                )

            # O^T accumulators, one PSUM bank per query half.
            o_ps = [
                apsum.tile([D, 512], F32, tag=f"o_ps{h}", name=f"o_ps{h}")
                for h in range(2)
            ]

            def kchunk(c):
                if c == 0:
                    return kT00_sb[:]
                blk, off = c // 8, (c % 8) * 128
                return kT_sb[blk][:, off : off + 128]

            def vchunk(c):
                blk, off = c // 8, (c % 8) * 128
                return vt_sb[blk][:, off : off + 128]

            def emit_qk(c):
                s_ps = spsum.tile([D, HW], F32, tag="s", name=f"s_ps{c}")
                for h in range(2):
                    nc.tensor.matmul(
                        s_ps[:, h * 512 : (h + 1) * 512],
                        kchunk(c),
                        qT_sb[:, h * 512 : (h + 1) * 512],
                        start=True,
                        stop=True,
                    )
                return s_ps

            # e-producer, emitted right after the chunk's QK so the engine
            # starts the moment the PSUM tile is ready.
            def emit_e(c, s_ps):
                if c in SCHR:
                    e_i16 = epool.tile([D, HW], I16, tag="e", name=f"e{c}")
                    nc.vector.tensor_scalar(
                        out=e_i16[:],
                        in0=s_ps[:],
                        scalar1=A7S,
                        scalar2=B7,
                        op0=AluOpType.mult,
                        op1=AluOpType.add,
                    )
                    return e_i16[:].bitcast(BF16)
                e_sb = epool.tile([D, HW], BF16, tag="e", name=f"e{c}")
                if c == NCHUNK - 1:
                    # Split halves so e63[:, 0:512] lands early enough for
                    # the pre-drain fold of the h0 denominator.
                    for h in range(2):
                        nc.scalar.activation(
                            e_sb[:, h * 512 : (h + 1) * 512],
                            s_ps[:, h * 512 : (h + 1) * 512],
                            mybir.ActivationFunctionType.Exp,
                            scale=SCALE,
                        )
                else:
                    nc.scalar.activation(
                        e_sb[:],
                        s_ps[:],
                        mybir.ActivationFunctionType.Exp,
                        scale=SCALE,
                    )
                return e_sb[:]

            rs_bc_ps = None
            fold_started = {0: False, 1: False}

            def emit_fold_h(src, h, final):
                nc.tensor.matmul(
                    rs_bc_ps[:, h * 512 : (h + 1) * 512],
                    ones16[:],
                    src[:, h * 512 : (h + 1) * 512],
                    start=not fold_started[h],
                    stop=final,
                )
                fold_started[h] = True

            def emit_rs(c, e16):
                reg = (c % 2) * HW
                region = rs2[:, reg : reg + HW]
                if c < 2:
                    nc.vector.tensor_copy(region, e16)
                elif c in POOL_RS:
                    nc.gpsimd.tensor_add(region, region, e16)
                else:
                    nc.vector.tensor_add(region, region, e16)

            def emit_av(c, e16, h):
                nc.tensor.matmul(
                    o_ps[h][:],
                    vchunk(c),
                    e16[:, h * 512 : (h + 1) * 512],
                    start=(c == 0),
                    stop=(c == NCHUNK - 1),
                )

            # Software-pipelined two chunks ahead (3 PSUM S-slots). The
            # rs-add for chunk c is emitted one slot late so a Schraudolph
            # exp never queues behind an rs-add that waits on a slow ACT exp
            # (DVE executes in program order).
            s_tiles = {0: emit_qk(0), 1: emit_qk(1)}
            e_tiles = {0: emit_e(0, s_tiles[0]), 1: emit_e(1, s_tiles[1])}
            e_done = {}
            for c in range(NCHUNK - 2):
                if c + 2 < NCHUNK:
                    s_tiles[c + 2] = emit_qk(c + 2)
                    e_tiles[c + 2] = emit_e(c + 2, s_tiles[c + 2])
                s_tiles.pop(c)
                e16 = e_done[c] = e_tiles.pop(c)
                emit_av(c, e16, 0)
                emit_av(c, e16, 1)
                if c >= 1:
                    emit_rs(c - 1, e_done.pop(c - 1))

            # chunk 62: AV, then fold the odd region (complete after rs61).
            e62 = e_tiles.pop(62)
            emit_av(62, e62, 0)
            emit_av(62, e62, 1)
            emit_rs(61, e_done.pop(61))
            emit_rs(62, e62)
            rs_bc_ps = spsum.tile([D, HW], F32, tag="s", name="rs_bc_ps")
            emit_fold_h(rs2[:, HW : 2 * HW], 0, final=False)
            emit_fold_h(rs2[:, HW : 2 * HW], 1, final=False)

            # chunk 63: per half, fold the even region and e63 directly,
            # then the final AV pair last so the denominator (and the DVE
            # reciprocal) completes before the PE drains.
            e63 = e_tiles.pop(63)
            for h in range(2):
                emit_fold_h(rs2[:, 0:HW], h, final=False)
                emit_fold_h(e63, h, final=True)
            emit_av(63, e63, 0)
            emit_av(63, e63, 1)

            # --- endgame: rs_bc_ps holds the full denominator replicated
            # across partitions; rs_bc[0:512] completes two matmuls before
            # the PE drains, so the DVE chain recip-h0 / mult-h0 / recip-h1 /
            # mult-h1 starts early and feeds two output DMAs.
            rec_sb = [
                persist.tile([D, 512], F32, tag=f"rec{i}", name=f"rec{i}")
                for i in range(2)
            ]
            o_sb = [
                persist.tile([D, 512], F32, tag=f"osb{h}", name=f"osb{h}")
                for h in range(2)
            ]
            for h in range(2):
                sl = slice(h * 512, (h + 1) * 512)
                nc.vector.reciprocal(rec_sb[h][:], rs_bc_ps[:, sl])
                nc.vector.tensor_tensor(
                    o_sb[h][:], o_ps[h][:], rec_sb[h][:], AluOpType.mult
                )
                nc.sync.dma_start(out=oT[:, sl], in_=o_sb[h][:])

    nc.compile()
    return nc


_NC_CACHE = None


def _get_nc():
    global _NC_CACHE
    if _NC_CACHE is None:
        _NC_CACHE = _build_nc()
    return _NC_CACHE


def _prep_inputs(x: np.ndarray) -> list[dict]:
    x = np.ascontiguousarray(x, dtype=np.float32)
    xr = x.reshape(B, C, HW)

    # K channel-major over all tokens: kT[d, b*1024+hw] = x[b, 128+d, hw]
    kT = np.ascontiguousarray(
        xr[:, 128:256, :].transpose(1, 0, 2)
    ).reshape(D, N).astype(ml_dtypes.bfloat16)
    kT00 = np.ascontiguousarray(kT[:, 0:128])
    # V chunk-transposed fp16: vt[p, 128*j + v] = V[128*j + p, v]
    v_tok = np.ascontiguousarray(xr[:, 0:128, :].transpose(0, 2, 1)).reshape(N, D)
    vt16 = np.ascontiguousarray(
        v_tok.reshape(NCHUNK, 128, D).transpose(1, 0, 2)
    ).reshape(D, N).astype(np.float16)

    in_maps = []
    for c in range(N_CORES):
        qT = np.ascontiguousarray(xr[c, 256:384, :]).astype(ml_dtypes.bfloat16)
        in_maps.append({"qT": qT, "kT00": kT00, "kT": kT, "vt16": vt16})
    return in_maps


def kernel(x: np.ndarray) -> np.ndarray:
    assert x.shape == (B, C, H, W), x.shape
    in_maps = _prep_inputs(x)
    nc = _get_nc()
    res = run_bass_kernel_spmd(nc, in_maps, list(range(N_CORES)))

    out = np.empty((B, D, H, W), dtype=np.float32)
    for c in range(N_CORES):
        out[c] = res.results[c]["oT"].reshape(D, H, W)
    return out


# revision 19
# speedup vs baseline: 1.0158x; 1.0158x over previous
"""Global-attention kernel for [8, 384, 32, 32] ConvAttention on 8 trn2 cores.

Math (per reference): tokens over B*H*W = 8192 positions, C = 384 channels
split as V/K/Q of 128 each; out = softmax(Q K^T / sqrt(128)) V, re-laid as
[B, 128, H, W].

Sharding: core c owns the 1024 query tokens of batch c (token n = b*1024+hw,
so batch == contiguous token block). K/V are replicated. Each core computes
its row block of the attention entirely locally; no collectives.

On-core layout: channel-major ([d, token]) everywhere, S^T formulation:
for each kv chunk j (128 tokens), S^T_j = K_j^T Q in PSUM (3 rotating
tiles), exp to bf16, then O^T += V_j^T E_j on PE, accumulated in two
half-tiles (cols 0:512 / 512:1024) so the first half drains one matmul
early.

Q/K are bf16 (halves input DMA vs f32; verified ~1e-2 rel err), V fp16.
The PE stream is the bottleneck (QK + AV = 2048 rows/chunk at 1 row/cycle,
2.4 GHz); every 4th chunk's exp runs as a Schraudolph bit-trick on Pool
(mostly) or DVE so the ACT engine (1038ns/exp vs 854ns chunk period) never
falls more than ~550ns behind and the PE never stalls on an e-tile or
PSUM-slot rotation.

Softmax denominator: bf16 adds on DVE into parity regions of rs2; odd
region folds (ones-matmul, partition-broadcast) after chunk 61, even after
62, and chunk 63's e folds directly, all before the last two AV matmuls so
the reciprocal (DVE) starts 426ns before the PE drains. Normalize+store is
pipelined in 4 query-column slices across DVE/Pool with 4 output DMAs.
"""

import math

import numpy as np
import ml_dtypes

import concourse.bass as bass
import concourse.tile as tile
from concourse import bacc, mybir
from concourse.alu_op_type import AluOpType
from concourse.bass_utils import run_bass_kernel_spmd

N_CORES = 8
B, C, H, W = 8, 384, 32, 32
HW = H * W            # 1024 tokens per batch == per core
N = B * HW            # 8192 total tokens
D = 128               # key/value width
NCHUNK = N // 128     # 64 kv chunks of 128 tokens
SCALE = 1.0 / math.sqrt(D)
F32 = mybir.dt.float32
F32R = mybir.dt.float32r
F16 = mybir.dt.float16
BF16 = mybir.dt.bfloat16
I16 = mybir.dt.int16

# Schraudolph exp (bf16 bit trick): i16 = x*a + b truncated to int16; the
# bit pattern read as bf16 approximates exp(x*SCALE) within ~3.5%.
A7S = float((1 << 7) / math.log(2.0) * SCALE)
B7 = float(127 * (1 << 7) - 6)

# Every 4th chunk's exp runs off the ACT engine (period-4 pattern keeps the
# ACT backlog bounded). GPSIMD cannot read PSUM on TRN2, so all of these
# run on DVE; to compensate, ~1 in 5 denominator adds (SBUF-only) moves to
# Pool.
SCHR = frozenset(c for c in range(NCHUNK) if c % 4 == 1)  # 16 chunks
POOL_RS = frozenset(c for c in range(2, 55) if c % 5 == 2)  # 11 chunks

N_WARMUP_MM = 13  # PE matmuls filling the pstate-ramp window while DMAs land


def _build_nc():
    nc = bacc.Bacc(
        "TRN2", target_bir_lowering=False, debug=False, num_devices=N_CORES
    )
    qT = nc.dram_tensor("qT", [D, HW], BF16, kind="ExternalInput").ap()
    kT00 = nc.dram_tensor("kT00", [D, D], BF16, kind="ExternalInput").ap()
    kT = nc.dram_tensor("kT", [D, N], BF16, kind="ExternalInput").ap()
    vt16 = nc.dram_tensor("vt16", [D, N], F16, kind="ExternalInput").ap()
    oT = nc.dram_tensor("oT", [D, HW], F32, kind="ExternalOutput").ap()

    with tile.TileContext(nc) as tc:
        with (
            tc.tile_pool(name="persist", bufs=1) as persist,
            tc.tile_pool(name="etile", bufs=10) as epool,
            tc.tile_pool(name="spsum", bufs=3, space="PSUM") as spsum,
            tc.tile_pool(name="apsum", bufs=1, space="PSUM") as apsum,
        ):
            # --- SBUF persistents ---
            qT_sb = persist.tile([D, HW], BF16, tag="qT_sb")
            kT00_sb = persist.tile([D, D], BF16, tag="kT00_sb")
            kT_sb = [
                persist.tile([D, HW], BF16, tag=f"kT{i}", name=f"kT_sb{i}")
                for i in range(8)
            ]
            vt_sb = [
                persist.tile([D, HW], F16, tag=f"vt{i}", name=f"vt_sb{i}")
                for i in range(8)
            ]
            # Dedicated head tiles: a reader (Ldweights) waits on every DMA
            # writing its tile, so the first chunks get their own tiles to
            # decouple from the bulk-block transfers.
            k0a_sb = persist.tile([D, 256], BF16, tag="k0a_sb")  # k chunks 1-2
            v00_sb = persist.tile([D, D], F16, tag="v00_sb")     # v chunk 0
            v01_sb = persist.tile([D, D], F16, tag="v01_sb")     # v chunk 1
            ones16 = persist.tile([D, D], BF16, tag="ones16")
            # Denominator partials: region 0 = even chunks, 1 = odd.
            rs2 = persist.tile([D, 2 * HW], BF16, tag="rs2")
            warm_sb = persist.tile([D, 256], F32, tag="warm_sb")

            nc.gpsimd.memset(warm_sb[:], 0.5)
            nc.gpsimd.memset(ones16[:], 1.0)

            # --- input DMAs. The HWDGE config stage serializes at ~625ns
            # per DMA, so order by first-use and split the head of V so the
            # first AV isn't stuck behind a full-block transfer.
            nc.sync.dma_start(out=kT00_sb[:], in_=kT00[:])
            nc.sync.dma_start(out=qT_sb[:, 0:512], in_=qT[:, 0:512])
            nc.sync.dma_start(out=qT_sb[:, 512:1024], in_=qT[:, 512:1024])
            nc.sync.dma_start(out=k0a_sb[:], in_=kT[:, 128:384])
            nc.sync.dma_start(out=v00_sb[:], in_=vt16[:, 0:128])
            nc.sync.dma_start(out=kT_sb[0][:, 384:1024], in_=kT[:, 384:1024])
            nc.sync.dma_start(out=v01_sb[:], in_=vt16[:, 128:256])
            nc.sync.dma_start(out=vt_sb[0][:, 256:1024], in_=vt16[:, 256:1024])
            for i in (1, 3, 5):
                nc.sync.dma_start(
                    out=kT_sb[i][:], in_=kT[:, i * HW : (i + 1) * HW]
                )
                nc.sync.dma_start(
                    out=kT_sb[i + 1][:], in_=kT[:, (i + 1) * HW : (i + 2) * HW]
                )
                nc.sync.dma_start(
                    out=vt_sb[i][:], in_=vt16[:, i * HW : (i + 1) * HW]
                )
                nc.sync.dma_start(
                    out=vt_sb[i + 1][:], in_=vt16[:, (i + 1) * HW : (i + 2) * HW]
                )
            nc.sync.dma_start(out=kT_sb[7][:], in_=kT[:, 7 * HW : 8 * HW])
            nc.sync.dma_start(out=vt_sb[7][:], in_=vt16[:, 7 * HW : 8 * HW])

            # --- PE warmup: occupy the sub-3us pstate window while the
            # first input DMAs land; results unused.
            wm_ps = spsum.tile([D, HW], F32, tag="s", name="warm_ps")
            for i in range(N_WARMUP_MM):
                nc.tensor.matmul(
                    wm_ps[:, 0:256],
                    warm_sb[:, 0:128].bitcast(F32R),
                    warm_sb[:].bitcast(F32R),
                    start=True,
                    stop=True,
                )

            # O^T accumulators, one PSUM bank per query half.
            o_ps = [
                apsum.tile([D, 512], F32, tag=f"o_ps{h}", name=f"o_ps{h}")
                for h in range(2)
            ]

            def kchunk(c):
                if c == 0:
                    return kT00_sb[:]
                if c in (1, 2):
                    return k0a_sb[:, (c - 1) * 128 : c * 128]
                blk, off = c // 8, (c % 8) * 128
                return kT_sb[blk][:, off : off + 128]

            def vchunk(c):
                if c == 0:
                    return v00_sb[:]
                if c == 1:
                    return v01_sb[:]
                blk, off = c // 8, (c % 8) * 128
                return vt_sb[blk][:, off : off + 128]

            def emit_qk(c):
                s_ps = spsum.tile([D, HW], F32, tag="s", name=f"s_ps{c}")
                for h in range(2):
                    nc.tensor.matmul(
                        s_ps[:, h * 512 : (h + 1) * 512],
                        kchunk(c),
                        qT_sb[:, h * 512 : (h + 1) * 512],
                        start=True,
                        stop=True,
                    )
                return s_ps

            # e-producer, emitted right after the chunk's QK so the engine
            # starts the moment the PSUM tile is ready.
            def emit_e(c, s_ps):
                if c in SCHR:
                    e_i16 = epool.tile([D, HW], I16, tag="e", name=f"e{c}")
                    nc.vector.tensor_scalar(
                        out=e_i16[:],
                        in0=s_ps[:],
                        scalar1=A7S,
                        scalar2=B7,
                        op0=AluOpType.mult,
                        op1=AluOpType.add,
                    )
                    return e_i16[:].bitcast(BF16)
                e_sb = epool.tile([D, HW], BF16, tag="e", name=f"e{c}")
                if c >= NCHUNK - 2:
                    # Split halves so e62/e63[:, 0:512] land early enough
                    # for the pre-drain folds of the h0 denominator.
                    for h in range(2):
                        nc.scalar.activation(
                            e_sb[:, h * 512 : (h + 1) * 512],
                            s_ps[:, h * 512 : (h + 1) * 512],
                            mybir.ActivationFunctionType.Exp,
                            scale=SCALE,
                        )
                else:
                    nc.scalar.activation(
                        e_sb[:],
                        s_ps[:],
                        mybir.ActivationFunctionType.Exp,
                        scale=SCALE,
                    )
                return e_sb[:]

            rs_bc_ps = None
            fold_started = {0: False, 1: False}

            def emit_fold_h(src, h, final):
                nc.tensor.matmul(
                    rs_bc_ps[:, h * 512 : (h + 1) * 512],
                    ones16[:],
                    src[:, h * 512 : (h + 1) * 512],
                    start=not fold_started[h],
                    stop=final,
                )
                fold_started[h] = True

            def emit_rs(c, e16):
                reg = (c % 2) * HW
                region = rs2[:, reg : reg + HW]
                if c < 2:
                    nc.vector.tensor_copy(region, e16)
                elif c in POOL_RS:
                    nc.gpsimd.tensor_add(region, region, e16)
                else:
                    nc.vector.tensor_add(region, region, e16)

            def emit_av(c, e16, h):
                nc.tensor.matmul(
                    o_ps[h][:],
                    vchunk(c),
                    e16[:, h * 512 : (h + 1) * 512],
                    start=(c == 0),
                    stop=(c == NCHUNK - 1),
                )

            # Software-pipelined two chunks ahead (3 PSUM S-slots). The
            # rs-add for chunk c is emitted two slots late so a Schraudolph
            # exp never queues behind an rs-add that waits on a slow ACT exp
            # (DVE executes in program order).
            s_tiles = {0: emit_qk(0), 1: emit_qk(1)}
            e_tiles = {0: emit_e(0, s_tiles[0]), 1: emit_e(1, s_tiles[1])}
            e_done = {}
            for c in range(NCHUNK - 2):
                if c + 2 < NCHUNK:
                    s_tiles[c + 2] = emit_qk(c + 2)
                    e_tiles[c + 2] = emit_e(c + 2, s_tiles[c + 2])
                s_tiles.pop(c)
                e16 = e_done[c] = e_tiles.pop(c)
                emit_av(c, e16, 0)
                emit_av(c, e16, 1)
                if c >= 2:
                    emit_rs(c - 2, e_done.pop(c - 2))

            # chunk 62: AV, remaining rs-adds (62 split by half so the even
            # fold isn't gated on the full-width add), then fold the odd
            # region (complete after rs61).
            e62 = e_tiles.pop(62)
            emit_av(62, e62, 0)
            emit_av(62, e62, 1)
            emit_rs(60, e_done.pop(60))
            emit_rs(61, e_done.pop(61))
            for h in range(2):
                sl = slice(h * 512, (h + 1) * 512)
                nc.vector.tensor_add(rs2[:, sl], rs2[:, sl], e62[:, sl])
            rs_bc_ps = spsum.tile([D, HW], F32, tag="s", name="rs_bc_ps")
            emit_fold_h(rs2[:, HW : 2 * HW], 0, final=False)
            emit_fold_h(rs2[:, HW : 2 * HW], 1, final=False)

            # chunk 63: per half, fold the even region and e63 directly,
            # then the final AV pair last so the denominator (and the DVE
            # reciprocal) completes before the PE drains.
            e63 = e_tiles.pop(63)
            for h in range(2):
                emit_fold_h(rs2[:, 0:HW], h, final=False)
                emit_fold_h(e63, h, final=True)
            emit_av(63, e63, 0)
            emit_av(63, e63, 1)

            # --- endgame: rs_bc_ps holds the full denominator replicated
            # across partitions; rs_bc[0:512] completes two matmuls before
            # the PE drains, so the DVE chain recip-h0 / mult-h0 / recip-h1 /
            # mult-h1 starts early and feeds two output DMAs.
            rec_sb = [
                persist.tile([D, 512], F32, tag=f"rec{i}", name=f"rec{i}")
                for i in range(2)
            ]
            o_sb = [
                persist.tile([D, 512], F32, tag=f"osb{h}", name=f"osb{h}")
                for h in range(2)
            ]
            for h in range(2):
                sl = slice(h * 512, (h + 1) * 512)
                nc.vector.reciprocal(rec_sb[h][:], rs_bc_ps[:, sl])
            for h in range(2):
                sl = slice(h * 512, (h + 1) * 512)
                nc.vector.tensor_tensor(
                    o_sb[h][:], o_ps[h][:], rec_sb[h][:], AluOpType.mult
                )
                nc.sync.dma_start(out=oT[:, sl], in_=o_sb[h][:])

    nc.compile()
    return nc


_NC_CACHE = None


def _get_nc():
    global _NC_CACHE
    if _NC_CACHE is None:
        _NC_CACHE = _build_nc()
    return _NC_CACHE


def _prep_inputs(x: np.ndarray) -> list[dict]:
    x = np.ascontiguousarray(x, dtype=np.float32)
    xr = x.reshape(B, C, HW)

    # K channel-major over all tokens: kT[d, b*1024+hw] = x[b, 128+d, hw]
    kT = np.ascontiguousarray(
        xr[:, 128:256, :].transpose(1, 0, 2)
    ).reshape(D, N).astype(ml_dtypes.bfloat16)
    kT00 = np.ascontiguousarray(kT[:, 0:128])
    # V chunk-transposed fp16: vt[p, 128*j + v] = V[128*j + p, v]
    v_tok = np.ascontiguousarray(xr[:, 0:128, :].transpose(0, 2, 1)).reshape(N, D)
    vt16 = np.ascontiguousarray(
        v_tok.reshape(NCHUNK, 128, D).transpose(1, 0, 2)
    ).reshape(D, N).astype(np.float16)

    in_maps = []
    for c in range(N_CORES):
        qT = np.ascontiguousarray(xr[c, 256:384, :]).astype(ml_dtypes.bfloat16)
        in_maps.append({"qT": qT, "kT00": kT00, "kT": kT, "vt16": vt16})
    return in_maps


def kernel(x: np.ndarray) -> np.ndarray:
    assert x.shape == (B, C, H, W), x.shape
    in_maps = _prep_inputs(x)
    nc = _get_nc()
    res = run_bass_kernel_spmd(nc, in_maps, list(range(N_CORES)))

    out = np.empty((B, D, H, W), dtype=np.float32)
    for c in range(N_CORES):
        out[c] = res.results[c]["oT"].reshape(D, H, W)
    return out


# revision 25
# speedup vs baseline: 1.0409x; 1.0247x over previous
"""Global-attention kernel for [8, 384, 32, 32] ConvAttention on 8 trn2 cores.

Math (per reference): tokens over B*H*W = 8192 positions, C = 384 channels
split as V/K/Q of 128 each; out = softmax(Q K^T / sqrt(128)) V, re-laid as
[B, 128, H, W].

Sharding: core c owns the 1024 query tokens of batch c (token n = b*1024+hw,
so batch == contiguous token block). K/V are replicated. Each core computes
its row block of the attention entirely locally; no collectives.

On-core layout: channel-major ([d, token]) everywhere, S^T formulation:
for each kv chunk j (128 tokens), S^T_j = K_j^T Q in PSUM (3 rotating
tiles), exp to bf16, then O^T += V_j^T E_j on PE, accumulated in two
half-tiles (cols 0:512 / 512:1024) so the first half drains one matmul
early.

Q/K are bf16 (halves input DMA vs f32; verified ~1e-2 rel err), V fp16.
The PE stream is the bottleneck (QK + AV = 2048 rows/chunk at 1 row/cycle,
2.4 GHz); every 4th chunk's exp runs as a Schraudolph bit-trick on Pool
(mostly) or DVE so the ACT engine (1038ns/exp vs 854ns chunk period) never
falls more than ~550ns behind and the PE never stalls on an e-tile or
PSUM-slot rotation.

Softmax denominator: bf16 adds on DVE into parity regions of rs2; odd
region folds (ones-matmul, partition-broadcast) after chunk 61, even after
62, and chunk 63's e folds directly, all before the last two AV matmuls so
the reciprocal (DVE) starts 426ns before the PE drains. Normalize+store is
pipelined in 4 query-column slices across DVE/Pool with 4 output DMAs.
"""

import math

import numpy as np
import ml_dtypes

import concourse.bass as bass
import concourse.tile as tile
from concourse import bacc, mybir
from concourse.alu_op_type import AluOpType
from concourse.bass_utils import run_bass_kernel_spmd

N_CORES = 8
B, C, H, W = 8, 384, 32, 32
HW = H * W            # 1024 tokens per batch == per core
N = B * HW            # 8192 total tokens
D = 128               # key/value width
NCHUNK = N // 128     # 64 kv chunks of 128 tokens
SCALE = 1.0 / math.sqrt(D)
F32 = mybir.dt.float32
F32R = mybir.dt.float32r
F16 = mybir.dt.float16
BF16 = mybir.dt.bfloat16
I16 = mybir.dt.int16

# Schraudolph exp (bf16 bit trick): i16 = x*a + b truncated to int16; the
# bit pattern read as bf16 approximates exp(x*SCALE) within ~3.5%.
A7S = float((1 << 7) / math.log(2.0) * SCALE)
B7 = float(127 * (1 << 7) - 6)

# Every 4th chunk's exp runs off the ACT engine (period-4 pattern keeps the
# ACT backlog bounded). GPSIMD cannot read PSUM on TRN2, so all of these
# run on DVE; to compensate, ~1 in 4 denominator adds (SBUF-only) moves to
# Pool.
SCHR = frozenset(c for c in range(NCHUNK) if c % 4 == 1)  # 16 chunks
POOL_RS = frozenset(range(14, 55, 4))  # 11 chunks
# The third exp of each ACT run (and the tail chunks) is emitted as two
# half-width calls so its first half meets the AV deadline even when the
# ACT chain is backlogged.
SPLIT_E = frozenset(c for c in range(4, NCHUNK) if c % 4 == 0) | {62, 63}

N_WARMUP_MM = 13  # PE matmuls filling the pstate-ramp window while DMAs land


def _build_nc():
    nc = bacc.Bacc(
        "TRN2", target_bir_lowering=False, debug=False, num_devices=N_CORES
    )
    qT = nc.dram_tensor("qT", [D, HW], BF16, kind="ExternalInput").ap()
    kT00 = nc.dram_tensor("kT00", [D, D], BF16, kind="ExternalInput").ap()
    kT = nc.dram_tensor("kT", [D, N], BF16, kind="ExternalInput").ap()
    vt16 = nc.dram_tensor("vt16", [D, N], F16, kind="ExternalInput").ap()
    oT = nc.dram_tensor("oT", [D, HW], F32, kind="ExternalOutput").ap()

    with tile.TileContext(nc) as tc:
        with (
            tc.tile_pool(name="persist", bufs=1) as persist,
            tc.tile_pool(name="etile", bufs=10) as epool,
            tc.tile_pool(name="spsum", bufs=3, space="PSUM") as spsum,
            tc.tile_pool(name="apsum", bufs=1, space="PSUM") as apsum,
        ):
            # --- SBUF persistents ---
            qT_sb = persist.tile([D, HW], BF16, tag="qT_sb")
            kT00_sb = persist.tile([D, D], BF16, tag="kT00_sb")
            kT_sb = [
                persist.tile([D, HW], BF16, tag=f"kT{i}", name=f"kT_sb{i}")
                for i in range(8)
            ]
            vt_sb = [
                persist.tile([D, HW], F16, tag=f"vt{i}", name=f"vt_sb{i}")
                for i in range(8)
            ]
            # Dedicated head tiles: a stationary-operand reader (Ldweights)
            # waits on every DMA writing its tile, so the first chunks get
            # their own tiles to decouple from the bulk-block transfers.
            k0a_sb = persist.tile([D, 256], BF16, tag="k0a_sb")  # k chunks 1-2
            k0b_sb = persist.tile([D, 256], BF16, tag="k0b_sb")  # k chunks 3-4
            v00_sb = persist.tile([D, D], F16, tag="v00_sb")     # v chunk 0
            v01_sb = persist.tile([D, D], F16, tag="v01_sb")     # v chunk 1
            ones16 = persist.tile([D, D], BF16, tag="ones16")
            # Denominator partials: region 0 = even chunks, 1 = odd.
            rs2 = persist.tile([D, 2 * HW], BF16, tag="rs2")
            warm_sb = persist.tile([D, 256], F32, tag="warm_sb")

            nc.gpsimd.memset(warm_sb[:], 0.5)
            nc.gpsimd.memset(ones16[:], 1.0)

            # --- input DMAs. The HWDGE config stage serializes at ~625ns
            # per DMA, so the SP chain carries only Q and K ordered by
            # first use; all of V goes through Pool's SWDGE path, which
            # bypasses HWDGE entirely.
            nc.sync.dma_start(out=kT00_sb[:], in_=kT00[:])
            nc.sync.dma_start(out=qT_sb[:, 0:512], in_=qT[:, 0:512])
            nc.sync.dma_start(out=qT_sb[:, 512:1024], in_=qT[:, 512:1024])
            nc.sync.dma_start(out=k0a_sb[:], in_=kT[:, 128:384])
            nc.sync.dma_start(out=k0b_sb[:], in_=kT[:, 384:640])
            nc.sync.dma_start(out=kT_sb[0][:, 640:1024], in_=kT[:, 640:1024])
            for i in range(1, 8):
                nc.sync.dma_start(
                    out=kT_sb[i][:], in_=kT[:, i * HW : (i + 1) * HW]
                )
            nc.gpsimd.dma_start(out=v00_sb[:], in_=vt16[:, 0:128])
            nc.gpsimd.dma_start(out=v01_sb[:], in_=vt16[:, 128:256])
            nc.gpsimd.dma_start(out=vt_sb[0][:, 256:1024], in_=vt16[:, 256:1024])
            for i in range(1, 8):
                nc.gpsimd.dma_start(
                    out=vt_sb[i][:], in_=vt16[:, i * HW : (i + 1) * HW]
                )

            # --- PE warmup: occupy the sub-3us pstate window while the
            # first input DMAs land; results unused.
            wm_ps = spsum.tile([D, HW], F32, tag="s", name="warm_ps")
            for i in range(N_WARMUP_MM):
                nc.tensor.matmul(
                    wm_ps[:, 0:256],
                    warm_sb[:, 0:128].bitcast(F32R),
                    warm_sb[:].bitcast(F32R),
                    start=True,
                    stop=True,
                )

            # O^T accumulators, one PSUM bank per query half.
            o_ps = [
                apsum.tile([D, 512], F32, tag=f"o_ps{h}", name=f"o_ps{h}")
                for h in range(2)
            ]

            def kchunk(c):
                if c == 0:
                    return kT00_sb[:]
                if c in (1, 2):
                    return k0a_sb[:, (c - 1) * 128 : c * 128]
                if c in (3, 4):
                    return k0b_sb[:, (c - 3) * 128 : (c - 2) * 128]
                blk, off = c // 8, (c % 8) * 128
                return kT_sb[blk][:, off : off + 128]

            def vchunk(c):
                if c == 0:
                    return v00_sb[:]
                if c == 1:
                    return v01_sb[:]
                blk, off = c // 8, (c % 8) * 128
                return vt_sb[blk][:, off : off + 128]

            def emit_qk(c):
                s_ps = spsum.tile([D, HW], F32, tag="s", name=f"s_ps{c}")
                for h in range(2):
                    nc.tensor.matmul(
                        s_ps[:, h * 512 : (h + 1) * 512],
                        kchunk(c),
                        qT_sb[:, h * 512 : (h + 1) * 512],
                        start=True,
                        stop=True,
                    )
                return s_ps

            # e-producer, emitted right after the chunk's QK so the engine
            # starts the moment the PSUM tile is ready.
            def emit_e(c, s_ps):
                if c in SCHR:
                    e_i16 = epool.tile([D, HW], I16, tag="e", name=f"e{c}")
                    nc.vector.tensor_scalar(
                        out=e_i16[:],
                        in0=s_ps[:],
                        scalar1=A7S,
                        scalar2=B7,
                        op0=AluOpType.mult,
                        op1=AluOpType.add,
                    )
                    return e_i16[:].bitcast(BF16)
                e_sb = epool.tile([D, HW], BF16, tag="e", name=f"e{c}")
                if c in SPLIT_E:
                    for h in range(2):
                        nc.scalar.activation(
                            e_sb[:, h * 512 : (h + 1) * 512],
                            s_ps[:, h * 512 : (h + 1) * 512],
                            mybir.ActivationFunctionType.Exp,
                            scale=SCALE,
                        )
                else:
                    nc.scalar.activation(
                        e_sb[:],
                        s_ps[:],
                        mybir.ActivationFunctionType.Exp,
                        scale=SCALE,
                    )
                return e_sb[:]

            rs_bc_ps = None
            fold_started = {0: False, 1: False}

            def emit_fold_h(src, h, final):
                nc.tensor.matmul(
                    rs_bc_ps[:, h * 512 : (h + 1) * 512],
                    ones16[:],
                    src[:, h * 512 : (h + 1) * 512],
                    start=not fold_started[h],
                    stop=final,
                )
                fold_started[h] = True

            def emit_rs(c, e16):
                reg = (c % 2) * HW
                region = rs2[:, reg : reg + HW]
                if c < 2:
                    nc.vector.tensor_copy(region, e16)
                elif c in POOL_RS:
                    nc.gpsimd.tensor_add(region, region, e16)
                else:
                    nc.vector.tensor_add(region, region, e16)

            def emit_av(c, e16, h):
                nc.tensor.matmul(
                    o_ps[h][:],
                    vchunk(c),
                    e16[:, h * 512 : (h + 1) * 512],
                    start=(c == 0),
                    stop=(c == NCHUNK - 1),
                )

            # Software-pipelined two chunks ahead (3 PSUM S-slots). Each
            # slot is [QK(c+2), AV(c-1) h1, AV(c) h0]: deferring the h1
            # matmul one slot relaxes its e-deadline by a full period, so a
            # late exp stalls at most the h0 half. The rs-add for chunk c
            # is emitted two slots late so a Schraudolph exp never queues
            # behind an rs-add that waits on a slow ACT exp (DVE executes
            # in program order).
            s_tiles = {0: emit_qk(0), 1: emit_qk(1)}
            e_tiles = {0: emit_e(0, s_tiles[0]), 1: emit_e(1, s_tiles[1])}
            e_done = {}
            for c in range(NCHUNK - 2):
                s_tiles[c + 2] = emit_qk(c + 2)
                e_tiles[c + 2] = emit_e(c + 2, s_tiles[c + 2])
                s_tiles.pop(c)
                e16 = e_done[c] = e_tiles.pop(c)
                if c >= 1:
                    emit_av(c - 1, e_done[c - 1], 1)
                emit_av(c, e16, 0)
                if c >= 2:
                    emit_rs(c - 2, e_done.pop(c - 2))

            # chunk 62: finish h1 of 61, h0 of 62, remaining rs-adds, then
            # fold the odd region (complete after rs61).
            e62 = e_tiles.pop(62)
            emit_av(61, e_done[61], 1)
            emit_av(62, e62, 0)
            emit_rs(60, e_done.pop(60))
            emit_rs(61, e_done.pop(61))
            rs_bc_ps = spsum.tile([D, HW], F32, tag="s", name="rs_bc_ps")
            emit_fold_h(rs2[:, HW : 2 * HW], 0, final=False)
            emit_fold_h(rs2[:, HW : 2 * HW], 1, final=False)

            # chunk 63: rs62 split by half so the even fold isn't gated on
            # the full-width add; per half fold the even region and e63
            # directly, then the final AV pair last so the denominator (and
            # the DVE reciprocal) completes before the PE drains.
            e63 = e_tiles.pop(63)
            emit_av(62, e62, 1)
            for h in range(2):
                sl = slice(h * 512, (h + 1) * 512)
                nc.vector.tensor_add(rs2[:, sl], rs2[:, sl], e62[:, sl])
            for h in range(2):
                emit_fold_h(rs2[:, 0:HW], h, final=False)
                emit_fold_h(e63, h, final=True)
            emit_av(63, e63, 0)
            emit_av(63, e63, 1)

            # --- endgame: rs_bc_ps holds the full denominator replicated
            # across partitions; rs_bc[0:512] completes two matmuls before
            # the PE drains, so the DVE chain recip-h0 / mult-h0 / recip-h1 /
            # mult-h1 starts early and feeds two output DMAs.
            rec_sb = [
                persist.tile([D, 512], F32, tag=f"rec{i}", name=f"rec{i}")
                for i in range(2)
            ]
            o_sb = [
                persist.tile([D, 512], F32, tag=f"osb{h}", name=f"osb{h}")
                for h in range(2)
            ]
            for h in range(2):
                sl = slice(h * 512, (h + 1) * 512)
                nc.vector.reciprocal(rec_sb[h][:], rs_bc_ps[:, sl])
            for h in range(2):
                sl = slice(h * 512, (h + 1) * 512)
                nc.vector.tensor_tensor(
                    o_sb[h][:], o_ps[h][:], rec_sb[h][:], AluOpType.mult
                )
                nc.sync.dma_start(out=oT[:, sl], in_=o_sb[h][:])

    nc.compile()
    return nc


_NC_CACHE = None


def _get_nc():
    global _NC_CACHE
    if _NC_CACHE is None:
        _NC_CACHE = _build_nc()
    return _NC_CACHE


def _prep_inputs(x: np.ndarray) -> list[dict]:
    x = np.ascontiguousarray(x, dtype=np.float32)
    xr = x.reshape(B, C, HW)

    # K channel-major over all tokens: kT[d, b*1024+hw] = x[b, 128+d, hw]
    kT = np.ascontiguousarray(
        xr[:, 128:256, :].transpose(1, 0, 2)
    ).reshape(D, N).astype(ml_dtypes.bfloat16)
    kT00 = np.ascontiguousarray(kT[:, 0:128])
    # V chunk-transposed fp16: vt[p, 128*j + v] = V[128*j + p, v]
    v_tok = np.ascontiguousarray(xr[:, 0:128, :].transpose(0, 2, 1)).reshape(N, D)
    vt16 = np.ascontiguousarray(
        v_tok.reshape(NCHUNK, 128, D).transpose(1, 0, 2)
    ).reshape(D, N).astype(np.float16)

    in_maps = []
    for c in range(N_CORES):
        qT = np.ascontiguousarray(xr[c, 256:384, :]).astype(ml_dtypes.bfloat16)
        in_maps.append({"qT": qT, "kT00": kT00, "kT": kT, "vt16": vt16})
    return in_maps


def kernel(x: np.ndarray) -> np.ndarray:
    assert x.shape == (B, C, H, W), x.shape
    in_maps = _prep_inputs(x)
    nc = _get_nc()
    res = run_bass_kernel_spmd(nc, in_maps, list(range(N_CORES)))

    out = np.empty((B, D, H, W), dtype=np.float32)
    for c in range(N_CORES):
        out[c] = res.results[c]["oT"].reshape(D, H, W)
    return out
